# revision 8
# baseline (speedup 1.0000x reference)
"""ChebGraphConv (K=3) Trainium2 kernel.

Reference computation (per batch b, time t; x0 = x[b,:,t,:].T in [N, C_IN]):
    out = x0 @ W0 + (G @ x0) @ W1 + (2 G (G @ x0) - x0) @ W2 + bias
Rewritten (G commutes with channel matmuls):
    P2  = x0 @ (2 W2)
    U   = G @ P2 + x0 @ W1
    out = G @ U  + x0 @ (W0 - W2) + bias

Sharding: batch B=16 split over 8 cores (2 b per core).  gso/weights
replicated.  All matmuls in float32r (fp32 rounded to 11 mantissa bits,
full-rate on the PE); inputs pre-rounded on host.

Per core loop over 8 blocks (b, blk) with 16 t's each:
  A[q]   = x[b, :, blk*16+4q : +4, :]  as [128=(c,d4), 1024=n]   (4 quads)
  S1: P2[nchunk 128, 512=(q,d,j)] = sum_c A[q]^T Wblk(2W2)        (block-diag)
  S2: U = G-matmuls (lhsT = gsoT chunks) + block-diag W1 accum
  S3: out = G-matmuls on U + block-diag (W0-W2) accum, + bias, DMA out
"""
import numpy as np

B, C_IN, T, N = 16, 32, 64, 1024
C_OUT = 32
N_CORES = 8
B_PER = B // N_CORES          # 2
N_BLK = T // 16               # 4 blocks of 16 t's per b -> 8 iters per core
NCH = N // 128                # 8 chunks of the node dim

_CACHE = {}


def _round_fp32r(x: np.ndarray) -> np.ndarray:
    """Round-to-nearest-even fp32 -> fp32r (11-bit mantissa), bit-exact with
    the hardware/neuron static_cast_fp32_to_fp32r."""
    b = np.ascontiguousarray(x, dtype=np.float32).view(np.uint32)
    lsb = (b >> 12) & 1
    return ((b + 0x7FF + lsb) & 0xFFFFF000).astype(np.uint32).view(np.float32)


def _split_multi_waits(nc, mybir, max_waits: int = 1):
    """Walrus rejects instructions whose ISA struct can't hold all their sync
    waits (fp32 self-loading matmul: 1).  Hoist excess waits onto inserted
    same-engine NoOps, which execute in order before the instruction."""
    import copy

    protos = {}

    def make_nop(engine, name):
        if engine not in protos:
            eng_map = {
                mybir.EngineType.SP: nc.sync,
                mybir.EngineType.PE: nc.tensor,
                mybir.EngineType.DVE: nc.vector,
                mybir.EngineType.Activation: nc.scalar,
                mybir.EngineType.Pool: nc.gpsimd,
            }
            proto = eng_map[engine].nop().ins
            for f in nc.m.functions:
                for blk in f.blocks:
                    insts = list(blk.instructions)
                    if insts and insts[-1] is proto:
                        insts.pop()
                        blk.instructions = insts
            protos[engine] = proto
        nop = copy.deepcopy(protos[engine])
        nop.name = name
        return nop

    for f in nc.m.functions:
        for blk in f.blocks:
            changed = False
            new = []
            for inst in blk.instructions:
                si = getattr(inst, "sync_info", None)
                waits = list(si.on_wait) if si is not None and si.on_wait else []
                if len(waits) > max_waits:
                    changed = True
                    extra, keep = waits[:-max_waits], waits[-max_waits:]
                    for k, w in enumerate(extra):
                        nop = make_nop(inst.engine, f"{inst.name}-hw{k}")
                        nop.sync_info = mybir.SyncInfo(on_wait=[w], on_update=[])
                        new.append(nop)
                    inst.sync_info = mybir.SyncInfo(
                        on_wait=keep, on_update=list(si.on_update or [])
                    )
                new.append(inst)
            if changed:
                blk.instructions = new


def _build_program():
    import concourse.bass as bass
    import concourse.mybir as mybir
    import concourse.tile as tile

    f32, f32r = mybir.dt.float32, mybir.dt.float32r
    bf16 = mybir.dt.bfloat16
    nc = bass.Bass("TRN2", target_bir_lowering=False, debug=False,
                   num_devices=N_CORES)

    xs_d = nc.dram_tensor("xs", [B_PER, C_IN, T, N], bf16, kind="ExternalInput")
    gt_d = nc.dram_tensor("gt", [N, N], bf16, kind="ExternalInput")
    wblk_d = nc.dram_tensor("wblk", [3, 128, 128], bf16, kind="ExternalInput")
    bias_d = nc.dram_tensor("biast", [128, 512], f32, kind="ExternalInput")
    out_d = nc.dram_tensor("out", [B_PER, T, N, C_OUT], f32, kind="ExternalOutput")

    with tile.TileContext(nc) as tc:
        with (
            tc.tile_pool(name="const", bufs=1) as cpool,
            tc.tile_pool(name="gt", bufs=1) as gtpool,
            tc.tile_pool(name="a", bufs=8) as apool,
            tc.tile_pool(name="p2g", bufs=16) as p2pool,
            tc.tile_pool(name="ug", bufs=12) as ugpool,
            tc.tile_pool(name="o", bufs=2) as opool,
            tc.tile_pool(name="ps1", bufs=4, space="PSUM") as ps1,
            tc.tile_pool(name="ps2", bufs=2, space="PSUM") as ps2,
            tc.tile_pool(name="ps3", bufs=2, space="PSUM") as ps3,
        ):
            # --- constants --- w22 rides the sync ring right behind A[0]
            # (stage1 needs it immediately); w1/w02/bias follow the first A
            # loads on the scalar ring (needed only by stages 2/3).
            def load_consts():
                wtiles = []
                for k, eng in zip(range(3), (nc.scalar, nc.sync, nc.scalar)):
                    w = cpool.tile([128, 128], bf16, tag=f"w{k}")
                    eng.dma_start(w[:], wblk_d.ap()[k])
                    wtiles.append(w)
                bias_sb = cpool.tile([128, 512], f32)
                nc.scalar.dma_start(bias_sb[:], bias_d.ap())
                return [t[:] for t in wtiles] + [bias_sb]
            def load_gt():
                # split across both HWDGE rings; emitted after the first A
                # loads so iteration 0's S1 isn't queued behind 4MB of G.
                gt_sb = []
                for ic in range(NCH):
                    g = gtpool.tile([128, N], bf16, tag=f"gt{ic}")
                    eng = nc.sync if ic % 2 == 0 else nc.scalar
                    eng.dma_start(g[:], gt_d.ap()[ic * 128:(ic + 1) * 128, :])
                    gt_sb.append(g)
                return gt_sb

            def load_A(b, blk):
                t0 = blk * 16
                A = []
                for q in range(4):
                    a = apool.tile([128, N], bf16, tag="a")
                    src = xs_d.ap()[b, :, t0 + 4 * q:t0 + 4 * q + 4, :]
                    eng = nc.sync if q % 2 == 0 else nc.scalar
                    eng.dma_start(a[:], src)  # order (c,t,n) == (p,n)
                    A.append(a)
                return A

            def stage1(A):
                """P2 = x0 @ (2 W2), laid out [n-chunk, (q,d,j)]."""
                p2g = []
                for nch in range(NCH):
                    ps = ps1.tile([128, 512], f32, tag="p2")
                    for q in range(4):
                        nc.tensor.matmul(
                            ps[:, q * 128:(q + 1) * 128],
                            A[q][:, nch * 128:(nch + 1) * 128],
                            w22_sb,
                            start=True, stop=True,
                        )
                    t = p2pool.tile([128, 512], bf16, tag="p2g")
                    nc.vector.tensor_copy(t[:], ps[:])
                    p2g.append(t)
                return p2g

            def stage2(A, p2g):
                """U = G @ P2 + x0 @ W1."""
                ug = []
                for nch in range(NCH):
                    ps = ps2.tile([128, 512], f32, tag="u")
                    for q in range(4):
                        nc.tensor.matmul(
                            ps[:, q * 128:(q + 1) * 128],
                            A[q][:, nch * 128:(nch + 1) * 128],
                            w1_sb,
                            start=True, stop=False,
                        )
                    for ic in range(NCH):
                        nc.tensor.matmul(
                            ps[:],
                            gt_sb[ic][:, nch * 128:(nch + 1) * 128],
                            p2g[ic][:],
                            start=False, stop=(ic == NCH - 1),
                        )
                    t = ugpool.tile([128, 512], bf16, tag="ug")
                    nc.vector.tensor_copy(t[:], ps[:])
                    ug.append(t)
                return ug

            def stage3(A, ug, b, blk):
                """out = G @ U + x0 @ (W0 - W2) + bias, then store."""
                t0 = blk * 16
                o_sb = opool.tile([128, NCH * 512], f32, tag="o")
                for hc in range(NCH):
                    ps = ps3.tile([128, 512], f32, tag="ou")
                    for q in range(4):
                        nc.tensor.matmul(
                            ps[:, q * 128:(q + 1) * 128],
                            A[q][:, hc * 128:(hc + 1) * 128],
                            w02_sb,
                            start=True, stop=False,
                        )
                    for ic in range(NCH):
                        nc.tensor.matmul(
                            ps[:],
                            gt_sb[ic][:, hc * 128:(hc + 1) * 128],
                            ug[ic][:],
                            start=False, stop=(ic == NCH - 1),
                        )
                    nc.vector.tensor_add(
                        o_sb[:, hc * 512:(hc + 1) * 512], ps[:], bias_sb[:]
                    )
                    # store this hc: src order (p, dt, j)
                    dst = out_d.ap()[b, t0:t0 + 16, hc * 128:(hc + 1) * 128, :]
                    eng = nc.sync if hc % 2 == 0 else nc.scalar
                    eng.dma_start(
                        dst.transpose([1, 0, 2]),
                        o_sb[:, hc * 512:(hc + 1) * 512],
                    )

            # --- software-pipelined main loop over 8 (b, blk) iterations ---
            iters = [(b, blk) for b in range(B_PER) for blk in range(N_BLK)]
            A_cur = load_A(*iters[0])
            w1_sb, w22_sb, w02_sb, bias_sb = load_consts()
            gt_sb = load_gt()
            p2g_cur = stage1(A_cur)
            for k in range(len(iters)):
                ug = stage2(A_cur, p2g_cur)
                if k + 1 < len(iters):
                    A_nxt = load_A(*iters[k + 1])
                    p2g_nxt = stage1(A_nxt)
                else:
                    A_nxt = p2g_nxt = None
                stage3(A_cur, ug, *iters[k])
                A_cur, p2g_cur = A_nxt, p2g_nxt

    _split_multi_waits(nc, mybir)
    return nc


def _prep_inputs(x, gso, weight, bias):
    """Host-side shard + repack.  Returns per-core in_maps."""
    import ml_dtypes

    w1, w22, w02 = weight[1], 2.0 * weight[2], weight[0] - weight[2]
    rows = (np.arange(C_IN)[None, :] * 4).repeat(4, 0) + np.arange(4)[:, None]
    wblk = np.zeros((3, 128, 128), np.float32)
    for k, w in enumerate((w1, w22, w02)):
        for d in range(4):
            wblk[k, rows[d], d * 32:(d + 1) * 32] = w
    wblk = wblk.astype(ml_dtypes.bfloat16)
    gt = np.ascontiguousarray(gso.T).astype(ml_dtypes.bfloat16)
    biast = np.tile(np.asarray(bias, np.float32), (128, 16))
    xb = np.asarray(x, np.float32).astype(ml_dtypes.bfloat16)
    in_maps = []
    for c in range(N_CORES):
        in_maps.append({
            "xs": np.ascontiguousarray(xb[c * B_PER:(c + 1) * B_PER]),
            "gt": gt,
            "wblk": wblk,
            "biast": biast,
        })
    return in_maps


def kernel(x, gso, weight, bias):
    from concourse import bass_utils

    x = np.asarray(x, np.float32)
    gso = np.asarray(gso, np.float32)
    weight = np.asarray(weight, np.float32)
    bias = np.asarray(bias, np.float32)

    if "nc" not in _CACHE:
        _CACHE["nc"] = _build_program()
    nc = _CACHE["nc"]

    in_maps = _prep_inputs(x, gso, weight, bias)
    res = bass_utils.run_bass_kernel_spmd(
        nc, in_maps, core_ids=list(range(N_CORES))
    )
    out = np.concatenate([r["out"] for r in res.results], axis=0)
    return out  # [B, T, N, C_OUT]

